# revision 9
# baseline (speedup 1.0000x reference)
"""Trainium2 Bass kernel for nn_FFTBias (FFT-based circular conv bias).

Math: the reference computes, per (batch, emb, head) signal x of length
S=4094 (zero-padded on the left by S-1):
    y = irfft(rfft_{8187}(x_pad) * rfft_{8187}(z_h), n=8186)[:S]
plus a normalizer z_pb from the same transform applied to the ones vector.

8187 = 3 * 2729 (2729 prime) and 8186 = 2 * 4093 (4093 prime), so an FFT
factorization is hopeless on this hardware — instead we evaluate the DFTs as
dense matmuls on the TensorEngine with host-precomputed (input-independent)
cos/sin matrices, padded to 4096x4096:

  forward:  P = Ca @ x, Q = Sa @ x          (freq x time matrices)
  per-head: Yr = P*Zr - Q*QzD, Yi = P*QzD + Q*Zr   (vector engine)
  inverse:  y = Cb @ Yr + Sb @ Yi           (time x freq matrices)

where Zr/QzD are the forward transform of rearranged w (folded via the
cos/sin symmetry so the SAME Ca/Sa matrices apply; the w[0] term is folded
in as an extra contraction row with CaT row = 1).

Sharding: 16 heads -> 2 heads per core x 8 cores (tensor parallel).  Per
core: 2 x 129 signal columns (128 v-signals + the ones column per head,
so the z_pb normalizer falls out of the same fused elementwise) + 4 aux
z columns = 262 rhs columns against 4 x (4096x4096) bf16 matmuls.
"""

import numpy as np
import ml_dtypes

import concourse.bass as bass
import concourse.tile as tile
from concourse import mybir
from concourse.bass import ts
from concourse.bass_utils import run_bass_kernel_spmd

S = 4094             # seq_len after dropping the 2 special tokens
N1 = 2 * S - 1       # 8187 rfft length
N2 = N1 - 1          # 8186 irfft default n
F = N1 // 2 + 1      # 4094 rfft bins
PAD = 4096           # padded transform dim (32 x 128)
B, H, E = 2, 16, 64
FULL_SEQ = 4096
NCORES = 8
HPC = H // NCORES    # heads per core = 2
NKT = PAD // 128     # 32 contraction tiles
NMB = PAD // 128     # 32 output row-blocks
HB = 128 + 1         # per-head block: 128 v-signals + 1 ones-column
NCOL = HPC * HB + 2 * HPC   # 262 forward rhs columns
NOUT = HPC * HB             # 258 inverse rhs columns

BF16 = mybir.dt.bfloat16
F32 = mybir.dt.float32

_cache = {}

# results of the most recent device run (exec_time_ns etc); for test harness
LAST_RUN = None


def _tile_weight(M):
    """(4096 contract, 4096 out) f32 -> (32 ob, 128 kp, 32*128) bf16 tiled
    so one out-block's weights are one contiguous 1MB chunk, laid out
    (kp, kt, o) per partition row."""
    t = M.reshape(NKT, 128, NMB, 128).transpose(2, 1, 0, 3).reshape(NMB, 128, NKT * 128)
    return t.astype(ml_dtypes.bfloat16)


def _build_weights():
    n = np.arange(PAD, dtype=np.int64)
    m = np.arange(PAD, dtype=np.int64)
    # exact angle reduction in int64 before cos/sin
    ang = (((n[:, None] + (S - 1)) * m[None, :]) % N1).astype(np.float64) * (2 * np.pi / N1)
    CaT = np.cos(ang)
    SaT = np.sin(ang)
    del ang
    CaT[S:, :] = 0.0
    SaT[S:, :] = 0.0
    CaT[S, :] = 1.0  # w0 correction row (contraction row S picks up w[h,0])

    ang2 = ((n[:, None] * m[None, :]) % N2).astype(np.float64) * (2 * np.pi / N2)
    ck = np.full(PAD, 2.0)
    ck[0] = 1.0
    ck[F - 1] = 1.0
    CbT = (ck[:, None] / N2) * np.cos(ang2)
    SbT = (2.0 / N2) * np.sin(ang2)
    del ang2
    CbT[F:, :] = 0.0  # kill padded junk freqs
    SbT[F:, :] = 0.0

    wf = np.ascontiguousarray(
        np.concatenate([_tile_weight(CaT), _tile_weight(SaT)], axis=2))
    wi = np.ascontiguousarray(
        np.concatenate([_tile_weight(CbT), _tile_weight(SbT)], axis=2))
    return wf, wi


def _split_dma_waits(nc):
    """This walrus build supports exactly ONE sync wait per instruction
    (setupSyncWait<...> asserts), but Tile's sem assignment can attach
    several (slot-reuse WAR/WAW + DMA-lane ordering).  Hoist all but one
    wait onto InstEventSemaphore instructions placed immediately before the
    instruction in the same engine stream — semantically identical (the
    sequencer blocks on them in order before issuing)."""
    uid = 0
    for blk in nc.m.functions[0].blocks:
        insts = blk.instructions
        i = 0
        while i < len(insts):
            inst = insts[i]
            si = inst.sync_info
            if si is not None and len(si.on_wait) > 1:
                waits = list(si.on_wait)
                for w in waits[:-1]:
                    ev = mybir.InstEventSemaphore(
                        name=f"EVW-{uid}", engine=inst.engine, ins=[], outs=[],
                        sync_info=mybir.SyncInfo(on_wait=[w], on_update=[]))
                    insts.insert(i, ev)
                    uid += 1
                    i += 1
                inst.sync_info = mybir.SyncInfo(
                    on_wait=[waits[-1]], on_update=list(si.on_update))
            i += 1


def _build_program():
    nc = bass.Bass("TRN2", target_bir_lowering=False, debug=False,
                   num_devices=NCORES)
    x_d = nc.dram_tensor("x", [128, NKT * NCOL], BF16, kind="ExternalInput").ap()
    wf_d = nc.dram_tensor("wf", [NMB, 128, 2 * NKT * 128], BF16, kind="ExternalInput").ap()
    wi_d = nc.dram_tensor("wi", [NMB, 128, 2 * NKT * 128], BF16, kind="ExternalInput").ap()
    out_d = nc.dram_tensor("out", [NMB, 128, NOUT], F32, kind="ExternalOutput").ap()

    MUL = mybir.AluOpType.mult
    ADD = mybir.AluOpType.add
    SUB = mybir.AluOpType.subtract

    with tile.TileContext(nc) as tc:
        with (
            tc.tile_pool(name="xp", bufs=1) as xp,
            tc.tile_pool(name="wfp", bufs=4) as wfp,
            tc.tile_pool(name="wip", bufs=3) as wip,
            tc.tile_pool(name="yp", bufs=1) as yp,
            tc.tile_pool(name="tmp", bufs=3) as tmpp,
            tc.tile_pool(name="aux", bufs=3) as auxpl,
            tc.tile_pool(name="outp", bufs=3) as outp,
            tc.tile_pool(name="pq", bufs=2, space="PSUM") as pqp,
            tc.tile_pool(name="ypsp", bufs=2, space="PSUM") as ypsp,
        ):
            xt = xp.tile([128, NKT * NCOL], BF16, name="xt")
            nc.sync.dma_start(out=xt, in_=x_d)
            # bf16 Yr / Yi buffers for the inverse transforms
            yr = yp.tile([128, NMB, NOUT], BF16, name="yr")
            yi = yp.tile([128, NMB, NOUT], BF16, name="yi")

            # ---- forward transforms + per-head complex multiply ----
            for mb in range(NMB):
                wt = wfp.tile([128, 2 * NKT * 128], BF16, name="wt")
                (nc.gpsimd if mb % 2 else nc.sync).dma_start(out=wt, in_=wf_d[mb])
                pps = pqp.tile([128, NCOL], F32, name="pps")
                qps = pqp.tile([128, NCOL], F32, name="qps")
                for kt in range(NKT):
                    nc.tensor.matmul(pps, lhsT=wt[:, ts(kt, 128)],
                                     rhs=xt[:, ts(kt, NCOL)],
                                     start=(kt == 0), stop=(kt == NKT - 1))
                for kt in range(NKT):
                    nc.tensor.matmul(qps, lhsT=wt[:, NKT * 128 + kt * 128:
                                                  NKT * 128 + (kt + 1) * 128],
                                     rhs=xt[:, ts(kt, NCOL)],
                                     start=(kt == 0), stop=(kt == NKT - 1))
                auxp = auxpl.tile([128, 2 * HPC], F32, name="auxp")
                auxq = auxpl.tile([128, 2 * HPC], F32, name="auxq")
                nc.vector.tensor_copy(out=auxp, in_=pps[:, HPC * HB:NCOL])
                nc.vector.tensor_copy(out=auxq, in_=qps[:, HPC * HB:NCOL])
                for h in range(HPC):
                    cs = slice(h * HB, (h + 1) * HB)
                    zr = auxp[:, 2 * h:2 * h + 1]      # Zr_h   (fp32)
                    qzd = auxq[:, 2 * h + 1:2 * h + 2]  # Q@zD_h (fp32); Zi = -QzD
                    t2 = tmpp.tile([128, HB], F32, name="t2")
                    nc.vector.tensor_scalar_mul(t2, qps[:, cs], qzd)
                    # Yr = P*Zr - Q*QzD
                    nc.vector.scalar_tensor_tensor(
                        out=yr[:, mb, cs], in0=pps[:, cs], scalar=zr, in1=t2,
                        op0=MUL, op1=SUB)
                    t4 = tmpp.tile([128, HB], F32, name="t4")
                    nc.vector.tensor_scalar_mul(t4, qps[:, cs], zr)
                    # Yi' = P*QzD + Q*Zr   (= -Yi; Sb sign folded)
                    nc.vector.scalar_tensor_tensor(
                        out=yi[:, mb, cs], in0=pps[:, cs], scalar=qzd, in1=t4,
                        op0=MUL, op1=ADD)

            # ---- inverse transforms ----
            for tb in range(NMB):
                wt2 = wip.tile([128, 2 * NKT * 128], BF16, name="wt2")
                (nc.gpsimd if tb % 2 else nc.sync).dma_start(out=wt2, in_=wi_d[tb])
                yps = ypsp.tile([128, NOUT], F32, name="ypsum")
                for mb in range(NMB):
                    nc.tensor.matmul(yps, lhsT=wt2[:, ts(mb, 128)],
                                     rhs=yr[:, mb, :],
                                     start=(mb == 0), stop=False)
                for mb in range(NMB):
                    nc.tensor.matmul(yps, lhsT=wt2[:, NKT * 128 + mb * 128:
                                                   NKT * 128 + (mb + 1) * 128],
                                     rhs=yi[:, mb, :],
                                     start=False, stop=(mb == NMB - 1))
                ot = outp.tile([128, NOUT], F32, name="ot")
                nc.vector.tensor_copy(out=ot, in_=yps)
                nc.sync.dma_start(out=out_d[tb], in_=ot)
    _split_dma_waits(nc)
    return nc


def _get_cached():
    if "wf" not in _cache:
        _cache["wf"], _cache["wi"] = _build_weights()
    if "nc" not in _cache:
        _cache["nc"] = _build_program()
    return _cache["nc"], _cache["wf"], _cache["wi"]


def _build_x_core(v_, w, o_, core):
    """Per-core forward rhs: (4096, 262) f32 -> tiled (128, 32*262) bf16."""
    X = np.zeros((PAD, NCOL), np.float32)
    for hl in range(HPC):
        h = HPC * core + hl
        X[:S, hl * HB:hl * HB + 128] = (
            v_[:, :, h, :].transpose(1, 0, 2).reshape(S, B * E))
        X[:S, hl * HB + 128] = o_
        wh = w[0, h].astype(np.float32)
        zA = wh[S - 1:]
        zB = np.zeros(S, np.float32)
        zB[2:] = wh[1:S - 1][::-1]
        X[:S, HPC * HB + 2 * hl] = zA + zB
        X[S, HPC * HB + 2 * hl] = wh[0]
        X[:S, HPC * HB + 2 * hl + 1] = zA - zB
    Xt = X.reshape(NKT, 128, NCOL).transpose(1, 0, 2).reshape(128, NKT * NCOL)
    return np.ascontiguousarray(Xt).astype(ml_dtypes.bfloat16)


def kernel(v, w, o_, **_trace_kwargs):
    global LAST_RUN
    v = np.asarray(v)
    w = np.asarray(w)
    o_ = np.asarray(o_)
    nc, wf, wi = _get_cached()

    v_ = v[:, 1:-1]  # (B, S, H, E)
    in_maps = [
        {"x": _build_x_core(v_, w, o_, c), "wf": wf, "wi": wi}
        for c in range(NCORES)
    ]
    res = run_bass_kernel_spmd(nc, in_maps, list(range(NCORES)), **_trace_kwargs)
    LAST_RUN = res

    pbv = np.zeros((B, FULL_SEQ, H, E), np.float32)
    zpb = np.zeros((1, FULL_SEQ, H), np.float32)
    for c in range(NCORES):
        y = np.asarray(res.results[c]["out"], np.float32).reshape(PAD, NOUT)
        for hl in range(HPC):
            h = HPC * c + hl
            blk = y[:S, hl * HB:hl * HB + 128]  # (S, 128)
            pbv[:, 1:1 + S, h, :] = blk.reshape(S, B, E).transpose(1, 0, 2)
            zpb[0, 1:1 + S, h] = y[:S, hl * HB + 128]
    return pbv, zpb


# revision 10
# speedup vs baseline: 1.0615x; 1.0615x over previous
"""Trainium2 Bass kernel for nn_FFTBias (FFT-based circular conv bias).

Math: the reference computes, per (batch, emb, head) signal x of length
S=4094 (zero-padded on the left by S-1):
    y = irfft(rfft_{8187}(x_pad) * rfft_{8187}(z_h), n=8186)[:S]
plus a normalizer z_pb from the same transform applied to the ones vector.

8187 = 3 * 2729 (2729 prime) and 8186 = 2 * 4093 (4093 prime), so an FFT
factorization is hopeless on this hardware — instead we evaluate the DFTs as
dense matmuls on the TensorEngine with host-precomputed (input-independent)
cos/sin matrices, padded to 4096x4096:

  forward:  P = Ca @ x, Q = Sa @ x          (freq x time matrices)
  per-head: Yr = P*Zr - Q*QzD, Yi = P*QzD + Q*Zr   (vector engine)
  inverse:  y = Cb @ Yr + Sb @ Yi           (time x freq matrices)

where Zr/QzD are the forward transform of rearranged w (folded via the
cos/sin symmetry so the SAME Ca/Sa matrices apply; the w[0] term is folded
in as an extra contraction row with CaT row = 1).

Sharding: 16 heads -> 2 heads per core x 8 cores (tensor parallel).  Per
core: 2 x 129 signal columns (128 v-signals + the ones column per head,
so the z_pb normalizer falls out of the same fused elementwise) + 4 aux
z columns = 262 rhs columns against 4 x (4096x4096) bf16 matmuls.
"""

import numpy as np
import ml_dtypes

import concourse.bass as bass
import concourse.tile as tile
from concourse import mybir
from concourse.bass import ts
from concourse.bass_utils import run_bass_kernel_spmd

S = 4094             # seq_len after dropping the 2 special tokens
N1 = 2 * S - 1       # 8187 rfft length
N2 = N1 - 1          # 8186 irfft default n
F = N1 // 2 + 1      # 4094 rfft bins
PAD = 4096           # padded transform dim (32 x 128)
B, H, E = 2, 16, 64
FULL_SEQ = 4096
NCORES = 8
HPC = H // NCORES    # heads per core = 2
NKT = PAD // 128     # 32 contraction tiles
NMB = PAD // 128     # 32 output row-blocks
HB = 128 + 1         # per-head block: 128 v-signals + 1 ones-column
NCOL = HPC * HB + 2 * HPC   # 262 forward rhs columns
NOUT = HPC * HB             # 258 inverse rhs columns

BF16 = mybir.dt.bfloat16
F32 = mybir.dt.float32

_cache = {}

# results of the most recent device run (exec_time_ns etc); for test harness
LAST_RUN = None


def _tile_weight(M):
    """(4096 contract, 4096 out) f32 -> (32 ob, 128 kp, 32*128) bf16 tiled
    so one out-block's weights are one contiguous 1MB chunk, laid out
    (kp, kt, o) per partition row."""
    t = M.reshape(NKT, 128, NMB, 128).transpose(2, 1, 0, 3).reshape(NMB, 128, NKT * 128)
    return t.astype(ml_dtypes.bfloat16)


def _build_weights():
    n = np.arange(PAD, dtype=np.int64)
    m = np.arange(PAD, dtype=np.int64)
    # exact angle reduction in int64 before cos/sin
    ang = (((n[:, None] + (S - 1)) * m[None, :]) % N1).astype(np.float64) * (2 * np.pi / N1)
    CaT = np.cos(ang)
    SaT = np.sin(ang)
    del ang
    CaT[S:, :] = 0.0
    SaT[S:, :] = 0.0
    CaT[S, :] = 1.0  # w0 correction row (contraction row S picks up w[h,0])

    ang2 = ((n[:, None] * m[None, :]) % N2).astype(np.float64) * (2 * np.pi / N2)
    ck = np.full(PAD, 2.0)
    ck[0] = 1.0
    ck[F - 1] = 1.0
    CbT = (ck[:, None] / N2) * np.cos(ang2)
    SbT = (2.0 / N2) * np.sin(ang2)
    del ang2
    CbT[F:, :] = 0.0  # kill padded junk freqs
    SbT[F:, :] = 0.0

    wf = np.ascontiguousarray(
        np.stack([_tile_weight(CaT), _tile_weight(SaT)], axis=1))
    wi = np.ascontiguousarray(
        np.stack([_tile_weight(CbT), _tile_weight(SbT)], axis=1))
    return wf, wi


def _split_dma_waits(nc):
    """This walrus build supports exactly ONE sync wait per instruction
    (setupSyncWait<...> asserts), but Tile's sem assignment can attach
    several (slot-reuse WAR/WAW + DMA-lane ordering).  Hoist all but one
    wait onto InstEventSemaphore instructions placed immediately before the
    instruction in the same engine stream — semantically identical (the
    sequencer blocks on them in order before issuing)."""
    uid = 0
    for blk in nc.m.functions[0].blocks:
        insts = blk.instructions
        i = 0
        while i < len(insts):
            inst = insts[i]
            si = inst.sync_info
            if si is not None and len(si.on_wait) > 1:
                waits = list(si.on_wait)
                for w in waits[:-1]:
                    ev = mybir.InstEventSemaphore(
                        name=f"EVW-{uid}", engine=inst.engine, ins=[], outs=[],
                        sync_info=mybir.SyncInfo(on_wait=[w], on_update=[]))
                    insts.insert(i, ev)
                    uid += 1
                    i += 1
                inst.sync_info = mybir.SyncInfo(
                    on_wait=[waits[-1]], on_update=list(si.on_update))
            i += 1


def _build_program():
    nc = bass.Bass("TRN2", target_bir_lowering=False, debug=False,
                   num_devices=NCORES)
    x_d = nc.dram_tensor("x", [128, NKT * NCOL], BF16, kind="ExternalInput").ap()
    wf_d = nc.dram_tensor("wf", [NMB, 2, 128, NKT * 128], BF16, kind="ExternalInput").ap()
    wi_d = nc.dram_tensor("wi", [NMB, 2, 128, NKT * 128], BF16, kind="ExternalInput").ap()
    out_d = nc.dram_tensor("out", [NMB, 128, NOUT], F32, kind="ExternalOutput").ap()

    MUL = mybir.AluOpType.mult
    ADD = mybir.AluOpType.add
    SUB = mybir.AluOpType.subtract

    with tile.TileContext(nc) as tc:
        with (
            tc.tile_pool(name="xp", bufs=1) as xp,
            tc.tile_pool(name="wfp", bufs=4) as wfp,
            tc.tile_pool(name="wip", bufs=3) as wip,
            tc.tile_pool(name="yp", bufs=1) as yp,
            tc.tile_pool(name="tmp", bufs=3) as tmpp,
            tc.tile_pool(name="aux", bufs=3) as auxpl,
            tc.tile_pool(name="outp", bufs=3) as outp,
            tc.tile_pool(name="pq", bufs=2, space="PSUM") as pqp,
            tc.tile_pool(name="ypsp", bufs=2, space="PSUM") as ypsp,
        ):
            xt = xp.tile([128, NKT * NCOL], BF16, name="xt")
            nc.sync.dma_start(out=xt, in_=x_d)
            # bf16 Yr / Yi buffers for the inverse transforms
            yr = yp.tile([128, NMB, NOUT], BF16, name="yr")
            yi = yp.tile([128, NMB, NOUT], BF16, name="yi")

            # ---- forward transforms + per-head complex multiply ----
            for mb in range(NMB):
                wtc = wfp.tile([128, NKT * 128], BF16, name="wtc")
                nc.gpsimd.dma_start(out=wtc, in_=wf_d[mb, 0])
                wts = wfp.tile([128, NKT * 128], BF16, name="wts")
                nc.gpsimd.dma_start(out=wts, in_=wf_d[mb, 1])
                pps = pqp.tile([128, NCOL], F32, name="pps")
                qps = pqp.tile([128, NCOL], F32, name="qps")
                for kt in range(NKT):
                    nc.tensor.matmul(pps, lhsT=wtc[:, ts(kt, 128)],
                                     rhs=xt[:, ts(kt, NCOL)],
                                     start=(kt == 0), stop=(kt == NKT - 1))
                for kt in range(NKT):
                    nc.tensor.matmul(qps, lhsT=wts[:, ts(kt, 128)],
                                     rhs=xt[:, ts(kt, NCOL)],
                                     start=(kt == 0), stop=(kt == NKT - 1))
                auxp = auxpl.tile([128, 2 * HPC], F32, name="auxp")
                auxq = auxpl.tile([128, 2 * HPC], F32, name="auxq")
                nc.vector.tensor_copy(out=auxp, in_=pps[:, HPC * HB:NCOL])
                nc.vector.tensor_copy(out=auxq, in_=qps[:, HPC * HB:NCOL])
                for h in range(HPC):
                    cs = slice(h * HB, (h + 1) * HB)
                    zr = auxp[:, 2 * h:2 * h + 1]      # Zr_h   (fp32)
                    qzd = auxq[:, 2 * h + 1:2 * h + 2]  # Q@zD_h (fp32); Zi = -QzD
                    t2 = tmpp.tile([128, HB], F32, name="t2")
                    nc.vector.tensor_scalar_mul(t2, qps[:, cs], qzd)
                    # Yr = P*Zr - Q*QzD
                    nc.vector.scalar_tensor_tensor(
                        out=yr[:, mb, cs], in0=pps[:, cs], scalar=zr, in1=t2,
                        op0=MUL, op1=SUB)
                    t4 = tmpp.tile([128, HB], F32, name="t4")
                    nc.vector.tensor_scalar_mul(t4, qps[:, cs], zr)
                    # Yi' = P*QzD + Q*Zr   (= -Yi; Sb sign folded)
                    nc.vector.scalar_tensor_tensor(
                        out=yi[:, mb, cs], in0=pps[:, cs], scalar=qzd, in1=t4,
                        op0=MUL, op1=ADD)

            # ---- inverse transforms ----
            for tb in range(NMB):
                wtc2 = wip.tile([128, NKT * 128], BF16, name="wtc2")
                nc.gpsimd.dma_start(out=wtc2, in_=wi_d[tb, 0])
                wts2 = wip.tile([128, NKT * 128], BF16, name="wts2")
                nc.gpsimd.dma_start(out=wts2, in_=wi_d[tb, 1])
                yps = ypsp.tile([128, NOUT], F32, name="ypsum")
                for mb in range(NMB):
                    nc.tensor.matmul(yps, lhsT=wtc2[:, ts(mb, 128)],
                                     rhs=yr[:, mb, :],
                                     start=(mb == 0), stop=False)
                for mb in range(NMB):
                    nc.tensor.matmul(yps, lhsT=wts2[:, ts(mb, 128)],
                                     rhs=yi[:, mb, :],
                                     start=False, stop=(mb == NMB - 1))
                ot = outp.tile([128, NOUT], F32, name="ot")
                nc.vector.tensor_copy(out=ot, in_=yps)
                nc.sync.dma_start(out=out_d[tb], in_=ot)
    _split_dma_waits(nc)
    return nc


def _get_cached():
    if "wf" not in _cache:
        _cache["wf"], _cache["wi"] = _build_weights()
    if "nc" not in _cache:
        _cache["nc"] = _build_program()
    return _cache["nc"], _cache["wf"], _cache["wi"]


def _build_x_core(v_, w, o_, core):
    """Per-core forward rhs: (4096, 262) f32 -> tiled (128, 32*262) bf16."""
    X = np.zeros((PAD, NCOL), np.float32)
    for hl in range(HPC):
        h = HPC * core + hl
        X[:S, hl * HB:hl * HB + 128] = (
            v_[:, :, h, :].transpose(1, 0, 2).reshape(S, B * E))
        X[:S, hl * HB + 128] = o_
        wh = w[0, h].astype(np.float32)
        zA = wh[S - 1:]
        zB = np.zeros(S, np.float32)
        zB[2:] = wh[1:S - 1][::-1]
        X[:S, HPC * HB + 2 * hl] = zA + zB
        X[S, HPC * HB + 2 * hl] = wh[0]
        X[:S, HPC * HB + 2 * hl + 1] = zA - zB
    Xt = X.reshape(NKT, 128, NCOL).transpose(1, 0, 2).reshape(128, NKT * NCOL)
    return np.ascontiguousarray(Xt).astype(ml_dtypes.bfloat16)


def kernel(v, w, o_, **_trace_kwargs):
    global LAST_RUN
    v = np.asarray(v)
    w = np.asarray(w)
    o_ = np.asarray(o_)
    nc, wf, wi = _get_cached()

    v_ = v[:, 1:-1]  # (B, S, H, E)
    in_maps = [
        {"x": _build_x_core(v_, w, o_, c), "wf": wf, "wi": wi}
        for c in range(NCORES)
    ]
    res = run_bass_kernel_spmd(nc, in_maps, list(range(NCORES)), **_trace_kwargs)
    LAST_RUN = res

    pbv = np.zeros((B, FULL_SEQ, H, E), np.float32)
    zpb = np.zeros((1, FULL_SEQ, H), np.float32)
    for c in range(NCORES):
        y = np.asarray(res.results[c]["out"], np.float32).reshape(PAD, NOUT)
        for hl in range(HPC):
            h = HPC * c + hl
            blk = y[:S, hl * HB:hl * HB + 128]  # (S, 128)
            pbv[:, 1:1 + S, h, :] = blk.reshape(S, B, E).transpose(1, 0, 2)
            zpb[0, 1:1 + S, h] = y[:S, hl * HB + 128]
    return pbv, zpb


# revision 11
# speedup vs baseline: 1.3822x; 1.3021x over previous
"""Trainium2 Bass kernel for nn_FFTBias (FFT-based circular conv bias).

Math: the reference computes, per (batch, emb, head) signal x of length
S=4094 (zero-padded on the left by S-1):
    y = irfft(rfft_{8187}(x_pad) * rfft_{8187}(z_h), n=8186)[:S]
plus a normalizer z_pb from the same transform applied to the ones vector.

8187 = 3 * 2729 (2729 prime) and 8186 = 2 * 4093 (4093 prime), so an FFT
factorization is hopeless on this hardware — instead we evaluate the DFTs as
dense matmuls on the TensorEngine with host-precomputed (input-independent)
cos/sin matrices:

  forward:  P = Ca @ x, Q = Sa @ x          (freq x time matrices, 4096 pad)
  per-head: Yr = P*Zr - Q*QzD, Yi = P*QzD + Q*Zr   (vector engine)
  inverse:  y = Cb @ Yr + Sb @ Yi           (time x freq)

The inverse is HALVED via the fold cos(2pi(4093-k)t/8186) = (-1)^t cos(.),
sin(2pi(4093-k)t/8186) = -(-1)^t sin(.): frequency pairs (k, 4093-k) are
combined on the vector engine (Re/Ro/Ie/Io), then even/odd output samples
each use a 2047-long contraction.  The forward frequency axis is emitted in
a permuted order (low half k=0..2046, high half 4093-k) so fold pairs sit
on the same SBUF partition — the permutation is free (host-generated
weights; the z-spectrum scalars ride through the same matmuls so they are
consistently permuted).

Zr/QzD come from the forward transform of rearranged w, folded via cos/sin
symmetry so the SAME Ca/Sa matrices apply; the w[0] DC term is folded in as
an extra contraction row whose CaT entries are 1.

Sharding: 16 heads -> 2 heads per core x 8 cores (tensor parallel).  Per
core: 2 x 129 signal columns (128 v-signals + a ones-column per head, so
the z_pb normalizer falls out of the same fused elementwise) + 4 aux z
columns = 262 forward rhs columns.
"""

import numpy as np
import ml_dtypes

import concourse.bass as bass
import concourse.tile as tile
from concourse import mybir
from concourse.bass import ts
from concourse.bass_utils import run_bass_kernel_spmd

S = 4094             # seq_len after dropping the 2 special tokens
N1 = 2 * S - 1       # 8187 rfft length
N2 = N1 - 1          # 8186 irfft default n
F = N1 // 2 + 1      # 4094 rfft bins
PAD = 4096           # padded forward transform dim (32 x 128)
HALF = 2048          # folded contraction / per-parity output dim (16 x 128)
K2 = 2047            # real folded freqs (pairs (k, 4093-k), k = 0..2046)
B, H, E = 2, 16, 64
FULL_SEQ = 4096
NCORES = 8
HPC = H // NCORES    # heads per core = 2
NKT = PAD // 128     # 32 forward contraction tiles
NMB = PAD // 128     # 32 forward output row-blocks
NKB = HALF // 128    # 16 folded contraction tiles
HB = 128 + 1         # per-head block: 128 v-signals + 1 ones-column
NCOL = HPC * HB + 2 * HPC   # 262 forward rhs columns
NOUT = HPC * HB             # 258 inverse rhs columns

BF16 = mybir.dt.bfloat16
F32 = mybir.dt.float32

_cache = {}

# results of the most recent device run (exec_time_ns etc); for test harness
LAST_RUN = None


def _perm_freqs():
    """Forward output (freq) order: col m<2048 -> freq m (m<=2046; 2047 pad),
    col m>=2048 -> freq 4093-(m-2048) (pad at m'=2047).  Pad cols -> -1."""
    p = np.full(PAD, -1, np.int64)
    p[:K2] = np.arange(K2)
    p[HALF:HALF + K2] = 4093 - np.arange(K2)
    return p


def _tile_w(M, nkt, nob):
    """(contract, out) f32 -> (nob, 128, nkt*128) bf16, per-out-block
    contiguous chunks laid out (kp, kt, o) per partition row."""
    t = (M.reshape(nkt, 128, nob, 128).transpose(2, 1, 0, 3)
          .reshape(nob, 128, nkt * 128))
    return t.astype(ml_dtypes.bfloat16)


def _build_weights():
    perm = _perm_freqs()
    valid = perm >= 0
    pf = np.where(valid, perm, 0)

    # ---- forward: CaT/SaT (time n=4096 x permuted freq m=4096) ----
    n = np.arange(PAD, dtype=np.int64)
    ang = (((n[:, None] + (S - 1)) * pf[None, :]) % N1).astype(np.float64) \
        * (2 * np.pi / N1)
    CaT = np.cos(ang) * valid[None, :]
    SaT = np.sin(ang) * valid[None, :]
    del ang
    CaT[S:, :] = 0.0
    SaT[S:, :] = 0.0
    CaT[S, :] = 1.0 * valid  # w0 correction row
    wf = np.ascontiguousarray(
        np.stack([_tile_w(CaT, NKT, NMB), _tile_w(SaT, NKT, NMB)], axis=1))
    del CaT, SaT

    # ---- inverse (folded): 4 matrices (2048 folded-k x 2048 tau) ----
    k = np.arange(HALF, dtype=np.int64)
    tau = np.arange(HALF, dtype=np.int64)
    g = np.full(HALF, 2.0 / N2)
    g[0] = 1.0 / N2          # pair (0, 4093) has c_k = 1
    g[K2:] = 0.0             # pad row
    sc = np.full(HALF, 2.0 / N2)
    sc[K2:] = 0.0

    def mk(tv):
        a = ((k[:, None] * tv[None, :]) % N2).astype(np.float64) \
            * (2 * np.pi / N2)
        return g[:, None] * np.cos(a), sc[:, None] * np.sin(a)

    Ce, Se = mk(2 * tau)        # even outputs t = 2 tau
    Co, So = mk(2 * tau + 1)    # odd outputs t = 2 tau + 1
    wi = np.empty((2 * NKB, 2, 128, NKB * 128), ml_dtypes.bfloat16)
    wi[:NKB, 0] = _tile_w(Ce, NKB, NKB)
    wi[:NKB, 1] = _tile_w(Se, NKB, NKB)
    wi[NKB:, 0] = _tile_w(Co, NKB, NKB)
    wi[NKB:, 1] = _tile_w(So, NKB, NKB)
    return wf, np.ascontiguousarray(wi)


def _split_dma_waits(nc):
    """This walrus build supports exactly ONE sync wait per instruction
    (setupSyncWait<...> asserts), but Tile's sem assignment can attach
    several (slot-reuse WAR/WAW + DMA-lane ordering).  Hoist all but one
    wait onto InstEventSemaphore instructions placed immediately before the
    instruction in the same engine stream — semantically identical (the
    sequencer blocks on them in order before issuing)."""
    uid = 0
    for blk in nc.m.functions[0].blocks:
        insts = blk.instructions
        i = 0
        while i < len(insts):
            inst = insts[i]
            si = inst.sync_info
            if si is not None and len(si.on_wait) > 1:
                waits = list(si.on_wait)
                for w in waits[:-1]:
                    ev = mybir.InstEventSemaphore(
                        name=f"EVW-{uid}", engine=inst.engine, ins=[], outs=[],
                        sync_info=mybir.SyncInfo(on_wait=[w], on_update=[]))
                    insts.insert(i, ev)
                    uid += 1
                    i += 1
                inst.sync_info = mybir.SyncInfo(
                    on_wait=[waits[-1]], on_update=list(si.on_update))
            i += 1


def _build_program():
    nc = bass.Bass("TRN2", target_bir_lowering=False, debug=False,
                   num_devices=NCORES)
    x_d = nc.dram_tensor("x", [128, NKT * NCOL], BF16, kind="ExternalInput").ap()
    wf_d = nc.dram_tensor("wf", [NMB, 2, 128, NKT * 128], BF16,
                          kind="ExternalInput").ap()
    wi_d = nc.dram_tensor("wi", [2 * NKB, 2, 128, NKB * 128], BF16,
                          kind="ExternalInput").ap()
    out_d = nc.dram_tensor("out", [2 * NKB, 128, NOUT], F32,
                           kind="ExternalOutput").ap()

    MUL = mybir.AluOpType.mult
    ADD = mybir.AluOpType.add
    SUB = mybir.AluOpType.subtract

    with tile.TileContext(nc) as tc:
        with (
            tc.tile_pool(name="xp", bufs=1) as xp,
            tc.tile_pool(name="wfp", bufs=3) as wfp,
            tc.tile_pool(name="wip", bufs=3) as wip,
            tc.tile_pool(name="yp", bufs=1) as yp,
            tc.tile_pool(name="fp", bufs=1) as fp,
            tc.tile_pool(name="tmp", bufs=3) as tmpp,
            tc.tile_pool(name="aux", bufs=3) as auxpl,
            tc.tile_pool(name="outp", bufs=3) as outp,
            tc.tile_pool(name="pq", bufs=2, space="PSUM") as pqp,
            tc.tile_pool(name="ypsp", bufs=2, space="PSUM") as ypsp,
        ):
            xt = xp.tile([128, NKT * NCOL], BF16, name="xt")
            nc.sync.dma_start(out=xt, in_=x_d)
            # Yr / Yi (permuted freq order) feeding the fold
            yr = yp.tile([128, NMB, NOUT], BF16, name="yr")
            yi = yp.tile([128, NMB, NOUT], BF16, name="yi")
            # folded inverse rhs: Re / Ro / Ie / Io
            fre = fp.tile([128, NKB, NOUT], BF16, name="fre")
            fro = fp.tile([128, NKB, NOUT], BF16, name="fro")
            fie = fp.tile([128, NKB, NOUT], BF16, name="fie")
            fio = fp.tile([128, NKB, NOUT], BF16, name="fio")

            # ---- forward transforms + per-head complex multiply ----
            # pair order (kb, kb+16) so the fold for pair kb runs early
            order = [kb + half for kb in range(NKB) for half in (0, NKB)]
            for mb in order:
                wtc = wfp.tile([128, NKT * 128], BF16, name="wtc")
                nc.gpsimd.dma_start(out=wtc, in_=wf_d[mb, 0])
                wts = wfp.tile([128, NKT * 128], BF16, name="wts")
                nc.gpsimd.dma_start(out=wts, in_=wf_d[mb, 1])
                pps = pqp.tile([128, NCOL], F32, name="pps")
                qps = pqp.tile([128, NCOL], F32, name="qps")
                for kt in range(NKT):
                    nc.tensor.matmul(pps, lhsT=wtc[:, ts(kt, 128)],
                                     rhs=xt[:, ts(kt, NCOL)],
                                     start=(kt == 0), stop=(kt == NKT - 1))
                for kt in range(NKT):
                    nc.tensor.matmul(qps, lhsT=wts[:, ts(kt, 128)],
                                     rhs=xt[:, ts(kt, NCOL)],
                                     start=(kt == 0), stop=(kt == NKT - 1))
                auxp = auxpl.tile([128, 2 * HPC], F32, name="auxp")
                auxq = auxpl.tile([128, 2 * HPC], F32, name="auxq")
                nc.vector.tensor_copy(out=auxp, in_=pps[:, HPC * HB:NCOL])
                nc.vector.tensor_copy(out=auxq, in_=qps[:, HPC * HB:NCOL])
                for h in range(HPC):
                    cs = slice(h * HB, (h + 1) * HB)
                    zr = auxp[:, 2 * h:2 * h + 1]       # Zr_h   (fp32)
                    qzd = auxq[:, 2 * h + 1:2 * h + 2]  # Q@zD_h; Zi = -QzD
                    t2 = tmpp.tile([128, HB], F32, name="t2")
                    nc.vector.tensor_scalar_mul(t2, qps[:, cs], qzd)
                    # Yr = P*Zr - Q*QzD
                    nc.vector.scalar_tensor_tensor(
                        out=yr[:, mb, cs], in0=pps[:, cs], scalar=zr, in1=t2,
                        op0=MUL, op1=SUB)
                    t4 = tmpp.tile([128, HB], F32, name="t4")
                    nc.vector.tensor_scalar_mul(t4, qps[:, cs], zr)
                    # Yi' = P*QzD + Q*Zr   (= -Yi; sin sign folded)
                    nc.vector.scalar_tensor_tensor(
                        out=yi[:, mb, cs], in0=pps[:, cs], scalar=qzd, in1=t4,
                        op0=MUL, op1=ADD)
                if mb >= NKB:
                    kb = mb - NKB
                    nc.vector.tensor_add(fre[:, kb, :], yr[:, kb, :],
                                         yr[:, kb + NKB, :])
                    nc.vector.tensor_sub(fro[:, kb, :], yr[:, kb, :],
                                         yr[:, kb + NKB, :])
                    nc.vector.tensor_sub(fie[:, kb, :], yi[:, kb, :],
                                         yi[:, kb + NKB, :])
                    nc.vector.tensor_add(fio[:, kb, :], yi[:, kb, :],
                                         yi[:, kb + NKB, :])

            # ---- folded inverse transforms (even blocks then odd) ----
            for ob in range(2 * NKB):
                rhs_c = fre if ob < NKB else fro
                rhs_s = fie if ob < NKB else fio
                wc2 = wip.tile([128, NKB * 128], BF16, name="wc2")
                nc.gpsimd.dma_start(out=wc2, in_=wi_d[ob, 0])
                ws2 = wip.tile([128, NKB * 128], BF16, name="ws2")
                nc.gpsimd.dma_start(out=ws2, in_=wi_d[ob, 1])
                yps = ypsp.tile([128, NOUT], F32, name="ypsum")
                for kt in range(NKB):
                    nc.tensor.matmul(yps, lhsT=wc2[:, ts(kt, 128)],
                                     rhs=rhs_c[:, kt, :],
                                     start=(kt == 0), stop=False)
                for kt in range(NKB):
                    nc.tensor.matmul(yps, lhsT=ws2[:, ts(kt, 128)],
                                     rhs=rhs_s[:, kt, :],
                                     start=False, stop=(kt == NKB - 1))
                ot = outp.tile([128, NOUT], F32, name="ot")
                nc.vector.tensor_copy(out=ot, in_=yps)
                nc.sync.dma_start(out=out_d[ob], in_=ot)
    _split_dma_waits(nc)
    return nc


def _get_cached():
    if "wf" not in _cache:
        _cache["wf"], _cache["wi"] = _build_weights()
    if "nc" not in _cache:
        _cache["nc"] = _build_program()
    return _cache["nc"], _cache["wf"], _cache["wi"]


def _build_x_core(v_, w, o_, core):
    """Per-core forward rhs: (4096, 262) f32 -> tiled (128, 32*262) bf16."""
    X = np.zeros((PAD, NCOL), np.float32)
    for hl in range(HPC):
        h = HPC * core + hl
        X[:S, hl * HB:hl * HB + 128] = (
            v_[:, :, h, :].transpose(1, 0, 2).reshape(S, B * E))
        X[:S, hl * HB + 128] = o_
        wh = w[0, h].astype(np.float32)
        zA = wh[S - 1:]
        zB = np.zeros(S, np.float32)
        zB[2:] = wh[1:S - 1][::-1]
        X[:S, HPC * HB + 2 * hl] = zA + zB
        X[S, HPC * HB + 2 * hl] = wh[0]
        X[:S, HPC * HB + 2 * hl + 1] = zA - zB
    Xt = X.reshape(NKT, 128, NCOL).transpose(1, 0, 2).reshape(128, NKT * NCOL)
    return np.ascontiguousarray(Xt).astype(ml_dtypes.bfloat16)


def kernel(v, w, o_, **_trace_kwargs):
    global LAST_RUN
    v = np.asarray(v)
    w = np.asarray(w)
    o_ = np.asarray(o_)
    nc, wf, wi = _get_cached()

    v_ = v[:, 1:-1]  # (B, S, H, E)
    in_maps = [
        {"x": _build_x_core(v_, w, o_, c), "wf": wf, "wi": wi}
        for c in range(NCORES)
    ]
    res = run_bass_kernel_spmd(nc, in_maps, list(range(NCORES)), **_trace_kwargs)
    LAST_RUN = res

    pbv = np.zeros((B, FULL_SEQ, H, E), np.float32)
    zpb = np.zeros((1, FULL_SEQ, H), np.float32)
    for c in range(NCORES):
        o = np.asarray(res.results[c]["out"], np.float32)
        ye = o[:NKB].reshape(HALF, NOUT)
        yo = o[NKB:].reshape(HALF, NOUT)
        y = np.empty((2 * HALF, NOUT), np.float32)
        y[0::2] = ye
        y[1::2] = yo
        for hl in range(HPC):
            h = HPC * c + hl
            blk = y[:S, hl * HB:hl * HB + 128]  # (S, 128)
            pbv[:, 1:1 + S, h, :] = blk.reshape(S, B, E).transpose(1, 0, 2)
            zpb[0, 1:1 + S, h] = y[:S, hl * HB + 128]
    return pbv, zpb
